# revision 2
# baseline (speedup 1.0000x reference)
"""Distributed GCN (2x GCNConv + Linear) on 8 Trainium2 NeuronCores via Bass/Tile.

v5 = v1 with packed gather chunks: per-(tile,section) buckets are laid out
contiguously inside each (group, section) chunk instead of being padded to
128-edge block multiples, cutting SWDGE descriptor-generation work (the
binding resource, ~4.3ns/row on the Pool engine) by ~20%.  Blocks that
straddle two dst tiles are matmul'd once per tile with the other tile's rows
zeroed in that tile's M panel (host-built).

Algorithm (matches the PyG-style reference):
  h1 = relu(gcnconv(x, W1, b1, mask1));  h2 = relu(gcnconv(h1, W2, b2, mask2))
  out = h2 @ Wl + bl
where gcnconv(x, W, b, keep) with self-loops:
  h = x @ W;  deg = segsum(keep, dst) + 1;  dis = rsqrt(deg)
  out = segsum(h[src] * (keep * dis[src] * dis[dst]), dst) + h * dis^2 + b

Distribution: nodes padded to N_PAD = 8 * SHARD, contiguous node shard per
core.  Edges partitioned by dst core.  Per layer: each core computes H for
its shard (TensorE), sectioned AllGather makes full H available in every
core's DRAM (bf16), per (7-tile group, section) one SWDGE dma_gather pulls
H[src] for the group's edges, M panels (host-built, coef-weighted one-hots)
stream from DRAM, and out^T[f, d] += G_blk^T @ M_panel accumulates on
TensorE in PSUM.  Self-loop blocks use the core's own shard rows scaled by
dis^2.  ReLU+bias on ScalarE out of PSUM; next layer's H-matmul per tile.
"""

import numpy as np
import ml_dtypes

import concourse.bass as bass
import concourse.bacc as bacc
import concourse.tile as tile
import concourse.mybir as mybir
from concourse.bass_utils import run_bass_kernel_spmd

P = 128
N_CORES = 8

N_NODES = 50000
F_IN = 128
F_HID = 128
F_OUT = 64

N_PAD = 50176
SHARD = N_PAD // N_CORES          # 6272
TILES_PC = SHARD // P             # 49
N_SEC = 2                         # src sections (int16 idx range + AG overlap)
SECL = SHARD // N_SEC             # 3136 local rows per section
GROUP_T = 7                       # dst tiles per group -> 7 groups
GATHER_BF16 = True
N_QUEUES = 4


# ---------------------------------------------------------------------------
# Host-side preprocessing
# ---------------------------------------------------------------------------

class _Chunk:
    __slots__ = ("half", "nblk", "blk0", "slot0", "caps")

    def __init__(self, half, nblk, blk0, slot0):
        self.half = half
        self.nblk = nblk
        self.blk0 = blk0          # global block offset
        self.slot0 = slot0        # global slot offset (= blk0 * P)
        self.caps = []            # (tt, base_in_chunk, cap)


class _Group:
    __slots__ = ("tiles", "p0", "p1")

    def __init__(self):
        # tt -> [(chunk_idx, j_in_chunk, panel_idx), ...]
        self.tiles = {}
        self.p0 = 0
        self.p1 = 0


class _LayerLayout:
    __slots__ = ("chunks", "groups", "n_blocks", "n_panels")

    def __init__(self):
        self.chunks = []
        self.groups = []
        self.n_blocks = 0
        self.n_panels = 0


def _prep_layer(src_k, dst_k, coef_k):
    """Packed layout: buckets contiguous within each (group, half) chunk."""
    s_all = src_k
    d_all = dst_k
    c_all = coef_k.astype(np.float32)

    tile_g = d_all // P
    half = (s_all % SHARD) // SECL
    key = tile_g * N_SEC + half
    order = np.argsort(key, kind="stable")
    s_all, d_all, c_all, key = s_all[order], d_all[order], c_all[order], key[order]
    s_idx = (s_all // SHARD) * SECL + (s_all % SHARD) % SECL
    bnd = np.searchsorted(key, np.arange(N_SEC * N_CORES * TILES_PC + 1))

    cnt = np.zeros((N_CORES, TILES_PC, N_SEC), dtype=np.int64)
    for t in range(N_CORES * TILES_PC):
        c, tt = divmod(t, TILES_PC)
        for h in range(N_SEC):
            cnt[c, tt, h] = bnd[N_SEC * t + h + 1] - bnd[N_SEC * t + h]
    cap = cnt.max(axis=0)                     # [TILES_PC, N_SEC]

    lay = _LayerLayout()
    groups = [list(range(g * GROUP_T, min((g + 1) * GROUP_T, TILES_PC)))
              for g in range(-(-TILES_PC // GROUP_T))]
    blk0 = 0
    panel = 0
    for tts in groups:
        grp = _Group()
        grp.p0 = panel
        for tt in tts:
            grp.tiles[tt] = []
        for h in range(N_SEC):
            base = 0
            ck = _Chunk(h, 0, blk0, blk0 * P)
            ci = len(lay.chunks)
            lay.chunks.append(ck)
            for tt in tts:
                cp = int(cap[tt, h])
                ck.caps.append((tt, base, cp))
                if cp > 0:
                    j0 = base // P
                    j1 = (base + cp - 1) // P
                    for j in range(j0, j1 + 1):
                        grp.tiles[tt].append((ci, j, panel))
                        panel += 1
                base += cp
            ck.nblk = -(-base // P)
            blk0 += ck.nblk
        grp.p1 = panel
        lay.groups.append(grp)
    lay.n_blocks = blk0
    lay.n_panels = panel

    # per-core arrays: wrapped idx16 + M panels
    per_core = []
    nblk = max(lay.n_blocks, 1)
    npan = max(lay.n_panels, 1)
    for c in range(N_CORES):
        idx16 = np.zeros((nblk * P,), dtype=np.int16)
        mbig = np.zeros((P, npan * P), dtype=np.float32)
        for grp in lay.groups:
            pass
        for ck in lay.chunks:
            for (tt, base, cp) in ck.caps:
                if cp == 0:
                    continue
                t = (0 * TILES_PC + tt)
                # global tile index for this core
                tg = c * TILES_PC + tt
                k = tg * N_SEC + ck.half
                a, b = bnd[k], bnd[k + 1]
                n_e = b - a
                assert n_e <= cp
                sl = slice(ck.slot0 + base, ck.slot0 + base + n_e)
                idx16[sl] = s_idx[a:b].astype(np.int16)
                # M entries filled below (need panel mapping)
        per_core.append({"idx16": idx16, "m": mbig})

    # panel fill: iterate groups/tiles/touched blocks
    for grp in lay.groups:
        for tt, touches in grp.tiles.items():
            for (ci, j, pan) in touches:
                ck = lay.chunks[ci]
                # slot range of this block
                s0 = ck.slot0 + j * P
                # bucket range for (tt, half) inside the chunk
                base_cp = [x for x in ck.caps if x[0] == tt]
                (_, base, cp) = base_cp[0]
                b0 = ck.slot0 + base
                for c in range(N_CORES):
                    tg = c * TILES_PC + tt
                    k = tg * N_SEC + ck.half
                    a, b = bnd[k], bnd[k + 1]
                    n_e = b - a
                    # edges of this core's bucket that land in block j
                    lo = max(s0, b0)
                    hi = min(s0 + P, b0 + n_e)
                    if lo >= hi:
                        continue
                    eo = lo - b0          # edge offset within bucket
                    cnt_e = hi - lo
                    rows = (lo - s0) + np.arange(cnt_e)   # partition rows
                    dl = (d_all[a + eo:a + eo + cnt_e] % P).astype(np.int64)
                    cf = c_all[a + eo:a + eo + cnt_e]
                    per_core[c]["m"][rows, pan * P + dl] = cf
    return lay, per_core


def _prepare(x, edge_index, mask1, mask2, W1, b1, W2, b2, Wl, bl,
             n=N_NODES, n_pad=N_PAD):
    assert n_pad == N_PAD
    src = np.asarray(edge_index[0], dtype=np.int64)
    dst = np.asarray(edge_index[1], dtype=np.int64)

    np_g = ml_dtypes.bfloat16 if GATHER_BF16 else np.float32

    layouts = []
    layer_data = []
    selfws = []
    for mask in (np.asarray(mask1), np.asarray(mask2)):
        keep = mask.astype(bool)
        ks, kd = src[keep], dst[keep]
        deg = np.bincount(kd, minlength=n).astype(np.float64) + 1.0
        dis = 1.0 / np.sqrt(deg)
        coef_k = (dis[ks] * dis[kd]).astype(np.float32)
        selfw = np.zeros((n_pad,), dtype=np.float32)
        selfw[:n] = (dis * dis).astype(np.float32)
        lay, pc = _prep_layer(ks, kd, coef_k)
        layouts.append(lay)
        layer_data.append(pc)
        selfws.append(selfw)

    xp = np.zeros((n_pad, F_IN), dtype=np.float32)
    xp[:n] = np.asarray(x, dtype=np.float32)

    ident = np.eye(P, dtype=np.float32)

    in_maps = []
    for c in range(N_CORES):
        m = {
            "xt": np.ascontiguousarray(xp[c * SHARD:(c + 1) * SHARD].T),
            "w1": np.asarray(W1, np.float32),
            "w2": np.asarray(W2, np.float32),
            "wl": np.asarray(Wl, np.float32),
            "b1c": np.asarray(b1, np.float32).reshape(P, 1),
            "b2c": np.asarray(b2, np.float32).reshape(P, 1),
            "blbc": np.broadcast_to(np.asarray(bl, np.float32),
                                    (P, F_OUT)).copy(),
            "ident": ident.astype(np_g),
        }
        for li in (0, 1):
            d = layer_data[li][c]
            w = d["idx16"].reshape(-1, 16).T
            m[f"idx{li+1}"] = np.ascontiguousarray(np.tile(w, (8, 1)))
            m[f"m{li+1}"] = d["m"].astype(np_g)
            sw = selfws[li][c * SHARD:(c + 1) * SHARD]
            m[f"sw{li+1}"] = np.ascontiguousarray(
                sw.reshape(TILES_PC, P).T.astype(np.float32))
        in_maps.append(m)
    return layouts, in_maps


# ---------------------------------------------------------------------------
# Device program
# ---------------------------------------------------------------------------

def _build(layouts, n_pad=N_PAD):
    assert n_pad == N_PAD
    gdt = mybir.dt.bfloat16 if GATHER_BF16 else mybir.dt.float32
    f32 = mybir.dt.float32

    nc = bacc.Bacc("TRN2", target_bir_lowering=False, debug=False,
                   num_swdge_queues=N_QUEUES)

    xt_d = nc.declare_dram_parameter("xt", [P, SHARD], f32, isOutput=False)
    w1_d = nc.declare_dram_parameter("w1", [P, F_HID], f32, isOutput=False)
    w2_d = nc.declare_dram_parameter("w2", [P, F_HID], f32, isOutput=False)
    wl_d = nc.declare_dram_parameter("wl", [P, F_OUT], f32, isOutput=False)
    b1c_d = nc.declare_dram_parameter("b1c", [P, 1], f32, isOutput=False)
    b2c_d = nc.declare_dram_parameter("b2c", [P, 1], f32, isOutput=False)
    blbc_d = nc.declare_dram_parameter("blbc", [P, F_OUT], f32, isOutput=False)
    ident_d = nc.declare_dram_parameter("ident", [P, P], gdt, isOutput=False)
    idx_d, m_d, sw_d = [], [], []
    for li, lay in enumerate(layouts):
        nb = max(lay.n_blocks, 1)
        npan = max(lay.n_panels, 1)
        idx_d.append(nc.declare_dram_parameter(
            f"idx{li+1}", [P, nb * 8], mybir.dt.int16, isOutput=False))
        m_d.append(nc.declare_dram_parameter(
            f"m{li+1}", [P, npan * P], gdt, isOutput=False))
        sw_d.append(nc.declare_dram_parameter(
            f"sw{li+1}", [P, TILES_PC], f32, isOutput=False))
    out_d = nc.declare_dram_parameter("out", [SHARD, F_OUT], f32, isOutput=True)

    h_shard = [nc.dram_tensor(f"h{li}_shard", [SHARD, P], gdt)
               for li in (1, 2)]
    h_sec = [[nc.dram_tensor(f"h{li}_sec{s}", [N_CORES * SECL, P], gdt,
                             addr_space="Shared") for s in range(N_SEC)]
             for li in (1, 2)]

    rg = [list(range(N_CORES))]
    relu = mybir.ActivationFunctionType.Relu
    copyf = mybir.ActivationFunctionType.Copy
    max_chunk_nb = max((ck.nblk for lay in layouts for ck in lay.chunks),
                      default=1)
    max_group_np = max((grp.p1 - grp.p0 for lay in layouts
                        for grp in lay.groups), default=1)
    qctr = [0]

    with tile.TileContext(nc) as tc:
        with (
            tc.tile_pool(name="consts", bufs=1) as cpool,
            tc.tile_pool(name="gbuf", bufs=8) as gpool,
            tc.tile_pool(name="mpool", bufs=3) as mpool,
            tc.tile_pool(name="spool", bufs=8) as spool,
            tc.tile_pool(name="opool", bufs=6) as opool,
            tc.tile_pool(name="aggp", bufs=5, space="PSUM") as aggpool,
            tc.tile_pool(name="hp", bufs=3, space="PSUM") as hpool,
        ):
            def load_const(dram, shape, dt):
                t = cpool.tile(shape, dt, tag=dram.name)
                nc.sync.dma_start(t[:], dram[:])
                return t

            xt_sb = load_const(xt_d, [P, SHARD], f32)
            w1_sb = load_const(w1_d, [P, F_HID], f32)
            w2_sb = load_const(w2_d, [P, F_HID], f32)
            wl_sb = load_const(wl_d, [P, F_OUT], f32)
            b1c_sb = load_const(b1c_d, [P, 1], f32)
            b2c_sb = load_const(b2c_d, [P, 1], f32)
            blbc_sb = load_const(blbc_d, [P, F_OUT], f32)
            ident_sb = load_const(ident_d, [P, P], gdt)
            idx_sb = [load_const(idx_d[li], [P, max(layouts[li].n_blocks, 1) * 8],
                                 mybir.dt.int16) for li in (0, 1)]
            sw_sb = [load_const(sw_d[li], [P, TILES_PC], f32) for li in (0, 1)]

            # ---- phase 0: H1 = X @ W1 (per-shard), sectioned AllGather ----
            for tt in range(TILES_PC):
                hp = hpool.tile([P, F_HID], f32, tag="hpsum")
                nc.tensor.matmul(out=hp[:], lhsT=xt_sb[:, tt * P:(tt + 1) * P],
                                 rhs=w1_sb[:], start=True, stop=True)
                hsb = opool.tile([P, F_HID], gdt, tag="hsb")
                nc.scalar.activation(out=hsb[:], in_=hp[:], func=copyf)
                nc.sync.dma_start(h_shard[0][tt * P:(tt + 1) * P, :], hsb[:])
                for s in range(N_SEC):
                    if tt * P < (s + 1) * SECL <= (tt + 1) * P:
                        nc.gpsimd.collective_compute(
                            "AllGather", mybir.AluOpType.bypass,
                            replica_groups=rg,
                            ins=[h_shard[0][s * SECL:(s + 1) * SECL, :]],
                            outs=[h_sec[0][s][:]])

            # ---- aggregation layers ----
            for li in (0, 1):
                lay = layouts[li]
                bcol = b1c_sb if li == 0 else b2c_sb
                w_next = w2_sb if li == 0 else wl_sb
                n_next = F_HID if li == 0 else F_OUT
                tcount = 0
                aggp = None

                for gi, grp in enumerate(lay.groups):
                    mw = mpool.tile([P, max_group_np * P], gdt, tag="mw")
                    nc.sync.dma_start(mw[:, :(grp.p1 - grp.p0) * P],
                                      m_d[li][:, grp.p0 * P:grp.p1 * P])
                    need = sorted({ci for touches in grp.tiles.values()
                                   for (ci, _, _) in touches})
                    gbufs = {}
                    for ci in need:
                        ck = lay.chunks[ci]
                        gb = gpool.tile([P, max_chunk_nb, P], gdt, tag="gb")
                        ni = ck.nblk * P
                        nc.gpsimd.dma_gather(
                            gb[:, :ck.nblk, :], h_sec[li][ck.half][:],
                            idx_sb[li][:, ck.blk0 * 8:ck.blk0 * 8 + ni // 16],
                            ni, ni, P, single_packet=False,
                            queue_num=qctr[0] % N_QUEUES)
                        qctr[0] += 1
                        gbufs[ci] = gb

                    for tt in sorted(grp.tiles.keys()):
                        if tcount % 4 == 0:
                            aggp = aggpool.tile([P, 512], f32, tag="aggp")
                        sl = slice((tcount % 4) * P, (tcount % 4) * P + P)
                        tcount += 1
                        bi = 0
                        for (ci, j, pan) in grp.tiles[tt]:
                            nc.tensor.matmul(
                                out=aggp[:, sl], lhsT=gbufs[ci][:, j, :],
                                rhs=mw[:, (pan - grp.p0) * P:
                                       (pan - grp.p0 + 1) * P],
                                start=(bi == 0), stop=False)
                            bi += 1
                        # self-loop block: own-shard H rows, scaled by dis^2
                        rows = slice(tt * P, (tt + 1) * P)
                        gs = spool.tile([P, P], gdt, tag="gself")
                        nc.sync.dma_start(gs[:], h_shard[li][rows, :])
                        gss = spool.tile([P, P], gdt, tag="gselfs")
                        nc.scalar.activation(out=gss[:], in_=gs[:], func=copyf,
                                             scale=sw_sb[li][:, tt:tt + 1])
                        nc.tensor.matmul(out=aggp[:, sl], lhsT=gss[:],
                                         rhs=ident_sb[:], start=(bi == 0),
                                         stop=True)
                        outT = opool.tile([P, P], f32, tag="outT")
                        nc.scalar.activation(out=outT[:], in_=aggp[:, sl],
                                             func=relu, bias=bcol[:])
                        hp2 = hpool.tile([P, F_HID], f32, tag="hpsum")
                        nc.tensor.matmul(out=hp2[:, :n_next], lhsT=outT[:],
                                         rhs=w_next[:], start=True, stop=True)
                        if li == 0:
                            hsb = opool.tile([P, F_HID], gdt, tag="hsb")
                            nc.scalar.activation(out=hsb[:], in_=hp2[:, :n_next],
                                                 func=copyf)
                            nc.sync.dma_start(h_shard[1][rows, :], hsb[:])
                            for s in range(N_SEC):
                                if tt * P < (s + 1) * SECL <= (tt + 1) * P:
                                    nc.gpsimd.collective_compute(
                                        "AllGather", mybir.AluOpType.bypass,
                                        replica_groups=rg,
                                        ins=[h_shard[1][s * SECL:
                                                        (s + 1) * SECL, :]],
                                        outs=[h_sec[1][s][:]])
                        else:
                            osb = opool.tile([P, F_OUT], f32, tag="osb")
                            nc.vector.tensor_tensor(
                                out=osb[:], in0=hp2[:, :n_next], in1=blbc_sb[:],
                                op=mybir.AluOpType.add)
                            nc.sync.dma_start(out_d[rows, :], osb[:])

    nc.compile()
    return nc


# ---------------------------------------------------------------------------
# Entry point
# ---------------------------------------------------------------------------

def kernel(x, edge_index, mask1, mask2, W1, b1, W2, b2, Wl, bl):
    layouts, in_maps = _prepare(x, edge_index, mask1, mask2,
                                W1, b1, W2, b2, Wl, bl)
    nc = _build(layouts)
    res = run_bass_kernel_spmd(nc, in_maps, core_ids=list(range(N_CORES)))
    out = np.concatenate([res.results[c]["out"] for c in range(N_CORES)],
                         axis=0)
    return out[:N_NODES].astype(np.float32)
